# revision 1
# baseline (speedup 1.0000x reference)
"""DIN attention kernel, data-parallel across 8 trn2 NeuronCores.

Shards the batch dim B=2048 across 8 cores (256 rows each); the tiny MLP
weights are replicated. Accepts FULL inputs, returns the FULL [B, D] output.
"""

import numpy as np
import jax
import jax.numpy as jnp

B, T, D = 2048, 200, 64
NEG_INF = -4294967295.0
M = 8  # cores


def _din_attention(query, key, mask, W1, b1, W2, b2, W3, b3):
    b, t, d = key.shape
    q_tile = jnp.broadcast_to(query[:, None, :], (b, t, d))
    din = jnp.concatenate([q_tile, key, q_tile - key, q_tile * key], axis=-1)
    h = jax.nn.sigmoid(jnp.einsum("btf,fh->bth", din, W1) + b1)
    h = jax.nn.sigmoid(jnp.einsum("bth,hg->btg", h, W2) + b2)
    score = (jnp.einsum("btg,go->bto", h, W3) + b3)[..., 0]
    key_mask = jnp.arange(t)[None, :] < mask[:, None]
    score = jnp.where(key_mask, score, NEG_INF)
    score = score / jnp.asarray(d, score.dtype) ** 0.5
    attn = jax.nn.softmax(score, axis=-1)
    return jnp.einsum("bt,btd->bd", attn, key)


_pfn = jax.pmap(
    _din_attention,
    in_axes=(0, 0, 0, None, None, None, None, None, None),
)


def kernel(query, key, mask, W1, b1, W2, b2, W3, b3):
    query = np.asarray(query, np.float32).reshape(M, B // M, D)
    key = np.asarray(key, np.float32).reshape(M, B // M, T, D)
    mask = np.asarray(mask, np.int32).reshape(M, B // M)
    out = _pfn(query, key, mask, W1, b1, W2, b2, W3, b3)
    return np.asarray(out).reshape(B, D).astype(np.float32)


# revision 2
# speedup vs baseline: 1.2022x; 1.2022x over previous
"""DIN attention kernel, data-parallel across 8 trn2 NeuronCores.

Shards the batch dim B=2048 across 8 cores (256 rows each); the tiny MLP
weights are replicated. Accepts FULL inputs, returns the FULL [B, D] output.
"""

import numpy as np
import jax
import jax.numpy as jnp

B, T, D = 2048, 200, 64
NEG_INF = -4294967295.0
M = 8  # cores


def _din_attention(query, key, mask, W1, b1, W2, b2, W3, b3):
    b, t, d = key.shape
    # din = [q, k, q-k, q*k]; fold the four D-blocks of W1 instead of
    # materializing the [b, t, 4D] concat:
    #   din @ W1 = q@(W1q+W1d) + k@(W1k-W1d) + (q*k)@W1m
    W1q, W1k, W1d, W1m = W1[:d], W1[d : 2 * d], W1[2 * d : 3 * d], W1[3 * d :]
    qpart = query @ (W1q + W1d) + b1                    # [b, H1]
    kpart = jnp.einsum("btd,dh->bth", key, W1k - W1d)   # [b, t, H1]
    mpart = jnp.einsum("btd,dh->bth", query[:, None, :] * key, W1m)
    h = jax.nn.sigmoid(qpart[:, None, :] + kpart + mpart)
    h = jax.nn.sigmoid(jnp.einsum("bth,hg->btg", h, W2) + b2)
    score = (jnp.einsum("btg,go->bto", h, W3) + b3)[..., 0]
    key_mask = jnp.arange(t)[None, :] < mask[:, None]
    score = jnp.where(key_mask, score, NEG_INF)
    score = score / jnp.asarray(d, score.dtype) ** 0.5
    attn = jax.nn.softmax(score, axis=-1)
    return jnp.einsum("bt,btd->bd", attn, key)


_pfn = jax.pmap(
    _din_attention,
    in_axes=(0, 0, 0, None, None, None, None, None, None),
)


def kernel(query, key, mask, W1, b1, W2, b2, W3, b3):
    query = np.asarray(query, np.float32).reshape(M, B // M, D)
    key = np.asarray(key, np.float32).reshape(M, B // M, T, D)
    mask = np.asarray(mask, np.int32).reshape(M, B // M)
    out = _pfn(query, key, mask, W1, b1, W2, b2, W3, b3)
    return np.asarray(out).reshape(B, D).astype(np.float32)
